# revision 63
# baseline (speedup 1.0000x reference)
"""Trainium2 Bass kernel for nn_ActorGraphPolicy (tree message-passing policy).

Pure data-parallel: batch 32768 sharded across 8 NeuronCores (4096 rows each).
Per-core program processes the batch in chunks of 512 columns, with all
activations kept feature-major ([feature, batch_cols]) in SBUF so every matmul
contracts over the partition dimension.

TRN2 engine ops require 32-aligned partition bases, so concatenated inputs use
padded layouts whose pad rows carry zero weights:
  cs tile [107, CB]: |dpos|@0, dpos@32, parent_pos@64, child_state@96 (11 rows)
  l1 input: xa1 = tanh(cs[0:107]) (rel part), xa2 = [tanh(mu); tanh(msg_in)]
"""
import os
import numpy as np

import concourse.bass as bass
import concourse.tile as tile
from concourse import bacc, mybir
from concourse.bass_utils import run_bass_kernel_spmd

AF = mybir.ActivationFunctionType
OP = mybir.AluOpType
F32 = mybir.dt.float32
F32R = mybir.dt.float32r

PARENTS = [-1, 0, 0, 1, 1, 2, 2, 3, 4, 5, 6, 7]
NL, SD, MD = 12, 11, 64
CHILDREN = [[i for i, p in enumerate(PARENTS) if p == n] for n in range(NL)]
SLOT = [PARENTS[:n].count(PARENTS[n]) for n in range(NL)]  # child slot index
BATCH = 32768
NCORES = 8
BLOC = BATCH // NCORES  # 4096
CB = 512                # batch columns per chunk
EPS = 1e-12

MM_DT = os.environ.get("MM_DT", "bf16")  # 'f32' | 'bf16'
NCH = int(os.environ.get("NCH", BLOC // CB))
SN_BUFS = int(os.environ.get("SN_BUFS", 1))
BF16 = mybir.dt.bfloat16
MDT = F32 if MM_DT == "f32" else BF16  # dtype of matmul-feeding tiles/weights

UP_ORDER = list(range(NL - 1, -1, -1))
# l1 output chunk layout (816 packed cols): j0-2 act[0:384], j3 = 48 rows
# [act 384:400 | 16 zeros | msg 384:400], j4-6 msg[0:384].
M_J = [128, 128, 128, 64, 128, 128, 128]
KS4 = [128, 128, 128, 32]
H2_KS = [128, 128, 45]


def _mm_in(ap):
    return ap


def build_program(nch=NCH):
    nc = bacc.Bacc("TRN2", target_bir_lowering=False)

    def din(name, shape):
        return nc.dram_tensor(name, shape, F32, kind="ExternalInput")

    def dinm(name, shape):
        return nc.dram_tensor(name, shape, MDT, kind="ExternalInput")

    statet = dinm("statet", [132, nch * CB])
    w1 = dinm("w1", [12, 64])  # row 11 = fc1 bias (SN row 11 == 1)
    wattp = dinm("wattp", [107, 64]); wattn = dinm("wattn", [107, 64])
    nbatt = din("nbatt", [64, 1])
    w2 = dinm("w2", [128, 65]); b2e = din("b2e", [65, 1])  # col64: h2 one-row
    w3 = dinm("w3", [65, 64])  # row 64 = fc3 bias (h2 row 64 == 1)
    ones64 = dinm("ones64", [64, 1]);   onesb64 = dinm("onesb64", [1, 64])
    ones128 = dinm("ones128", [128, 1]); onesb128 = dinm("onesb128", [1, 128])
    sel2 = dinm("sel2", [128, 2]); selb2 = dinm("selb2", [2, 128])
    wl1a = dinm("wl1a", [107, 832])   # rel rows (padded); row 35 = l1 biases
    wl1b = dinm("wl1b", [128, 832])   # [mu(64); msg_in(64)] rows
    wl2a = [dinm(f"wl2a{i}", [KS4[i], 301]) for i in range(4)]
    wl2m = [dinm(f"wl2m{i}", [64 if i == 3 else KS4[i], 301]) for i in range(4)]
    wl3a = [dinm(f"wl3a{i}", [H2_KS[i], 1]) for i in range(3)]
    wl3m = [dinm(f"wl3m{i}", [H2_KS[i], 128]) for i in range(3)]
    outt = nc.dram_tensor("outt", [12, nch * CB], F32, kind="ExternalOutput")

    with tile.TileContext(nc) as tc:
        with (
            nc.allow_low_precision(reason="bf16 matmul inputs; PSUM accumulates fp32"),
            tc.tile_pool(name="wp", bufs=1) as wp,          # weights, persistent
            tc.tile_pool(name="pp", bufs=1) as pp,          # per-chunk persistent
            tc.tile_pool(name="tp", bufs=1) as tp,          # transients
            tc.tile_pool(name="pbig", bufs=4, space="PSUM") as pbig,
            tc.tile_pool(name="pnrm", bufs=1, space="PSUM") as pnrm,
            tc.tile_pool(name="psm", bufs=3, space="PSUM") as psm,
        ):
            def wload(dram, shape, tag, dt=MDT):
                t = wp.tile(shape, dt, tag=tag, name=tag)
                nc.sync.dma_start(t[:], dram[:])
                return t

            W1 = wload(w1, [12, 64], "W1")
            WATTP = wload(wattp, [107, 64], "WATTP")
            WATTN = wload(wattn, [107, 64], "WATTN")
            NBATT = wload(nbatt, [64, 1], "NBATT", dt=F32)
            W2 = wload(w2, [128, 65], "W2"); B2E = wload(b2e, [65, 1], "B2E", dt=F32)
            W3 = wload(w3, [65, 64], "W3")
            ON64 = wload(ones64, [64, 1], "ON64");  OB64 = wload(onesb64, [1, 64], "OB64")
            ON128 = wload(ones128, [128, 1], "ON128"); OB128 = wload(onesb128, [1, 128], "OB128")
            SEL2 = wload(sel2, [128, 2], "SEL2")
            ONER = wp.tile([1, CB], MDT, tag="ONER", name="ONER")
            nc.gpsimd.memset(ONER[:], 1.0)
            TENR = wp.tile([1, CB], MDT, tag="TENR", name="TENR")
            nc.gpsimd.memset(TENR[:], 10.0)
            SELB2 = wload(selb2, [2, 128], "SELB2")
            WL1A = wload(wl1a, [107, 832], "WL1A"); WL1B = wload(wl1b, [128, 832], "WL1B")
            WL2A = [wload(wl2a[i], [KS4[i], 301], f"WL2A{i}") for i in range(4)]
            WL2M = [wload(wl2m[i], [64 if i == 3 else KS4[i], 301], f"WL2M{i}")
                    for i in range(4)]
            WL3A = [wload(wl3a[i], [H2_KS[i], 1], f"WL3A{i}") for i in range(3)]
            WL3M = [wload(wl3m[i], [H2_KS[i], 128], f"WL3M{i}") for i in range(3)]

            for c in range(nch):
                # ===== input: state arrives pre-transposed ([132, B]) ========
                SN = [pp.tile([SD + 1, CB], MDT, tag=f"sn{n}", name=f"sn{n}",
                              bufs=SN_BUFS) for n in range(NL)]
                ccols = slice(c * CB, (c + 1) * CB)
                for n in range(NL):
                    nc.sync.dma_start(SN[n][0:SD, :], statet[SD * n:SD * (n + 1), ccols])
                    if c < SN_BUFS:
                        nc.sync.dma_start(SN[n][SD:SD + 1, :], ONER[:])

                MU = [pp.tile([64, CB], F32, tag=f"mu{n}", name=f"mu{n}", bufs=2) for n in range(NL)]
                MD = [pp.tile([128, CB], F32, tag=f"md{n}", name=f"md{n}") for n in range(NL)]
                CS = {}   # child -> padded cs tile [107, CB]

                def l2norm(dst_ap, raw_ap, rows, ones_t, onesb_t):
                    sq = tp.tile([rows, CB], MDT, tag="sq", name="sq")
                    nc.vector.tensor_mul(sq[:], raw_ap, raw_ap)
                    pn = psm.tile([1, CB], F32, tag="S", name="pn")
                    nc.tensor.matmul(pn[:], _mm_in(ones_t[0:rows]), _mm_in(sq[:]),
                                     start=True, stop=True)
                    # 1/max(sqrt(ss), eps) == rsqrt(ss) for any nonzero vector
                    ninv = tp.tile([1, CB], MDT, tag="ninv", name="ninv")
                    nc.scalar.activation(ninv[:], pn[:], AF.Abs_reciprocal_sqrt)
                    pb = pnrm.tile([rows, CB], F32, tag="N", name="pb")
                    nc.tensor.matmul(pb[:], _mm_in(onesb_t[:, 0:rows]), _mm_in(ninv[:]),
                                     start=True, stop=True)
                    nc.vector.tensor_mul(dst_ap, raw_ap, pb[:])

                def build_cs(dst, nfrom, nto, with_rest):
                    """dst[0:3]=|d|, [32:35]=d=pos(nfrom)-pos(nto), [64:67]=pos(nto),
                    [96:96+r]=state(nfrom)."""
                    d3 = tp.tile([3, CB], MDT, tag="d3", name="d3")
                    nc.vector.tensor_sub(d3[:], SN[nfrom][0:3], SN[nto][0:3])
                    n3 = tp.tile([3, CB], MDT, tag="n3", name="n3")
                    nc.vector.tensor_scalar_mul(n3[:], d3[:], -1.0)
                    nc.vector.tensor_copy(dst[32:35], d3[:])
                    nc.vector.tensor_max(dst[0:3], d3[:], n3[:])
                    nc.vector.tensor_copy(dst[64:67], SN[nto][0:3])
                    nc.vector.tensor_copy(dst[96:96 + (SD if with_rest else 3)],
                                          SN[nfrom][0:SD if with_rest else 3])

                # ---- pre-pass: everything that depends only on state ----
                AT = {}
                for n_ in range(NL):
                    ch_ = CHILDREN[n_]
                    if not ch_:
                        continue
                    for c_i in ch_:
                        cst = pp.tile([107, CB], MDT, tag=f"cs{c_i}", name=f"cs{c_i}", bufs=2)
                        CS[c_i] = cst
                        if c < 2:
                            nc.gpsimd.memset(cst[:], 0.0)
                            nc.sync.dma_start(cst[35:36, :], TENR[:])
                        build_cs(cst, c_i, n_, True)
                    p_ = pbig.tile([64, CB], F32, tag="P", name="plpre")
                    if len(ch_) == 2:
                        nc.tensor.matmul(p_[:], _mm_in(WATTP[:]), _mm_in(CS[ch_[0]][:]),
                                         start=True, stop=False)
                        nc.tensor.matmul(p_[:], _mm_in(WATTN[:]), _mm_in(CS[ch_[1]][:]),
                                         start=False, stop=True)
                        at = tp.tile([64, CB], MDT, tag=f"at{n_}", name=f"at{n_}", bufs=2)
                        nc.scalar.activation(at[:], p_[:], AF.Sigmoid)
                    else:
                        nc.tensor.matmul(p_[:], _mm_in(WATTP[:]), _mm_in(CS[ch_[0]][:]),
                                         start=True, stop=True)
                        at = tp.tile([64, CB], MDT, tag=f"at{n_}", name=f"at{n_}", bufs=2)
                        nc.scalar.activation(at[:], p_[:], AF.Sigmoid, bias=NBATT[:])
                    AT[n_] = at
                rootcs = pp.tile([107, CB], MDT, tag="rootcs", name="rootcs", bufs=2)
                if c < 2:
                    nc.gpsimd.memset(rootcs[:], 0.0)
                    nc.sync.dma_start(rootcs[35:36, :], TENR[:])
                build_cs(rootcs, 0, NL - 1, False)

                def _up_node(n, xnp, mrp):
                    ch = CHILDREN[n]
                    r = 64 * (n % 2)
                    xm = tp.tile([128, CB], MDT, tag="xm", name="xm", bufs=2)
                    nc.scalar.activation(xm[0:64], xnp[r:r + 64], AF.Tanh)
                    if ch:
                        m = tp.tile([64, CB], F32, tag="m", name="m")
                        if len(ch) == 2:
                            # m = mu1 + sigmoid(l0 - l1) * (mu0 - mu1)
                            dmu = tp.tile([64, CB], F32, tag="dmu", name="dmu")
                            nc.vector.tensor_sub(dmu[:], MU[ch[0]][:], MU[ch[1]][:])
                            nc.vector.tensor_mul(dmu[:], AT[n][:], dmu[:])
                            nc.vector.tensor_add(m[:], dmu[:], MU[ch[1]][:])
                        else:
                            nc.vector.tensor_mul(m[:], AT[n][:], MU[ch[0]][:])
                        nc.scalar.activation(xm[64:128], m[:], AF.Tanh)
                        p2 = pbig.tile([65, CB], F32, tag="P", name="p2")
                        nc.tensor.matmul(p2[:], _mm_in(W2[:]), _mm_in(xm[:]),
                                         start=True, stop=True)
                    else:
                        p2 = pbig.tile([65, CB], F32, tag="P", name="p2")
                        nc.tensor.matmul(p2[:], _mm_in(W2[0:64]), _mm_in(xm[0:64]),
                                         start=True, stop=True)
                    h2 = tp.tile([65, CB], MDT, tag="h2u", name="h2u")
                    nc.scalar.activation(h2[:], p2[:], AF.Tanh, bias=B2E[:])
                    p3 = pbig.tile([64, CB], F32, tag="P", name="p3")
                    nc.tensor.matmul(p3[:], _mm_in(W3[:]), _mm_in(h2[:]),
                                     start=True, stop=True)
                    if mrp is not None:
                        nc.vector.tensor_copy(mrp[r:r + 64], p3[:])
                    else:
                        mr = tp.tile([64, CB], F32, tag="mr", name="mr")
                        nc.vector.tensor_copy(mr[:], p3[:])
                        l2norm(MU[n][:], mr[:], 64, ON64, OB64)

                # ================= bottom-up pass ============================
                # Nodes processed in pairs (11,10)...(1,0); x-norms and (where
                # the tree allows) mu-norms are computed 2-at-a-time through a
                # packed [128, CB] tile and a single sum/broadcast matmul pair.
                for na, nb in [(11, 10), (9, 8), (7, 6), (5, 4), (3, 2), (1, 0)]:
                    xhp = tp.tile([128, CB], F32, tag="xhp", name="xhp")
                    for n in (na, nb):
                        px = pbig.tile([64, CB], F32, tag="P", name="px")
                        nc.tensor.matmul(px[:], _mm_in(W1[:]), _mm_in(SN[n][:]),
                                         start=True, stop=True)
                        r = 64 * (n % 2)
                        nc.vector.tensor_copy(xhp[r:r + 64], px[:])
                    sqp = tp.tile([128, CB], MDT, tag="sqp", name="sqp")
                    nc.vector.tensor_mul(sqp[:], xhp[:], xhp[:])
                    pn2 = psm.tile([2, CB], F32, tag="S", name="pn2")
                    nc.tensor.matmul(pn2[:], _mm_in(SEL2[:]), _mm_in(sqp[:]),
                                     start=True, stop=True)
                    xinv2 = tp.tile([2, CB], MDT, tag="xinv2", name="xinv2")
                    nc.scalar.activation(xinv2[:], pn2[:], AF.Abs_reciprocal_sqrt)
                    pb2 = pnrm.tile([128, CB], F32, tag="N", name="pb2")
                    nc.tensor.matmul(pb2[:], _mm_in(SELB2[:]), _mm_in(xinv2[:]),
                                     start=True, stop=True)
                    xnp = tp.tile([128, CB], F32, tag="xnp", name="xnp")
                    nc.vector.tensor_mul(xnp[:], xhp[:], pb2[:])

                    mu_paired = (na, nb) != (1, 0)
                    if mu_paired:
                        mrp = tp.tile([128, CB], F32, tag="mrp", name="mrp")
                    for n in (na, nb):
                        _up_node(n, xnp, mrp if mu_paired else None)
                    if mu_paired:
                        sqm = tp.tile([128, CB], MDT, tag="sqm", name="sqm")
                        nc.vector.tensor_mul(sqm[:], mrp[:], mrp[:])
                        pnm = psm.tile([2, CB], F32, tag="S", name="pnm")
                        nc.tensor.matmul(pnm[:], _mm_in(SEL2[:]), _mm_in(sqm[:]),
                                         start=True, stop=True)
                        minv = tp.tile([2, CB], MDT, tag="minv", name="minv")
                        nc.scalar.activation(minv[:], pnm[:], AF.Abs_reciprocal_sqrt)
                        pbm = pnrm.tile([128, CB], F32, tag="N", name="pbm")
                        nc.tensor.matmul(pbm[:], _mm_in(SELB2[:]), _mm_in(minv[:]),
                                         start=True, stop=True)
                        for n in (na, nb):
                            r = 64 * (n % 2)
                            nc.vector.tensor_mul(MU[n][:], mrp[r:r + 64],
                                                 pbm[r:r + 64])


                # ================= top-down pass =============================
                for n in range(NL):
                    p = PARENTS[n]
                    rcs = rootcs if p < 0 else CS[n]
                    xa1 = tp.tile([107, CB], MDT, tag="xa1", name="xa1", bufs=2)
                    nc.scalar.activation(xa1[:], rcs[:], AF.Tanh)
                    xa2 = tp.tile([128, CB], MDT, tag="xa2", name="xa2", bufs=2)
                    nc.scalar.activation(xa2[0:64], MU[n][:], AF.Tanh)
                    if p >= 0:
                        mi = MD[p][64 * SLOT[n]: 64 * SLOT[n] + 64]
                        nc.scalar.activation(xa2[64:128], mi, AF.Tanh)
                        k2 = 128
                    else:
                        k2 = 64  # root: msg_in = 0 contributes nothing

                    h1 = []
                    col0 = 0
                    for j in range(7):
                        mj = M_J[j]
                        ph = pbig.tile([mj, CB], F32, tag="P", name="ph1")
                        cols = slice(col0, col0 + mj)
                        col0 += mj
                        nc.tensor.matmul(ph[:], _mm_in(WL1A[:, cols]), _mm_in(xa1[:]),
                                         start=True, stop=False)
                        nc.tensor.matmul(ph[:], _mm_in(WL1B[0:k2, cols]), _mm_in(xa2[0:k2]),
                                         start=False, stop=True)
                        h = tp.tile([mj, CB], MDT, tag=f"h1_{j}", name=f"h1_{j}", bufs=2)
                        nc.vector.tensor_single_scalar(h[:], ph[:], 0.0, OP.max)
                        h1.append(h)
                    aK = [h1[0][:], h1[1][:], h1[2][:], h1[3][0:32]]
                    mK = [h1[4][:], h1[5][:], h1[6][:], h1[3][32:64]]

                    h2a, h2m = [], []
                    for head, (Kp, WL2, acc) in enumerate(
                        ((aK, WL2A, h2a), (mK, WL2M, h2m))
                    ):
                        for i in range(3):
                            mi_ = H2_KS[i]
                            ph = pbig.tile([mi_, CB], F32, tag="P", name="ph2")
                            cols = slice(128 * i, 128 * i + mi_)  # i2: 45 cols incl one-row
                            for kk in range(4):
                                w_ap = WL2[kk][:, cols]
                                if head == 1 and kk == 3:
                                    w_ap = WL2[kk][32:64, cols]
                                nc.tensor.matmul(ph[:], _mm_in(w_ap), _mm_in(Kp[kk]),
                                                 start=(kk == 0), stop=(kk == 3))
                            h = tp.tile([mi_, CB], MDT, tag=f"h2_{head}_{i}",
                                        name=f"h2_{head}_{i}", bufs=2)
                            nc.scalar.activation(h[:], ph[:], AF.Relu)
                            acc.append(h)

                    pa = psm.tile([1, CB], F32, tag="S", name="pa")
                    for i in range(3):
                        nc.tensor.matmul(pa[:], _mm_in(WL3A[i][:]), _mm_in(h2a[i][:]),
                                         start=(i == 0), stop=(i == 2))
                    arow = tp.tile([1, CB], F32, tag="arow", name="arow")
                    nc.scalar.activation(arow[:], pa[:], AF.Tanh)
                    nc.sync.dma_start(outt[n:n + 1, ccols], arow[:])

                    pm = pbig.tile([128, CB], F32, tag="P", name="pm")
                    for i in range(3):
                        nc.tensor.matmul(pm[:], _mm_in(WL3M[i][:]), _mm_in(h2m[i][:]),
                                         start=(i == 0), stop=(i == 2))
                    mdr = tp.tile([128, CB], F32, tag="mdr", name="mdr")
                    nc.vector.tensor_copy(mdr[:], pm[:])
                    l2norm(MD[n][:], mdr[:], 128, ON128, OB128)

    nc.compile()
    return nc


def pack_inputs(inputs, shard):
    """Build the in_map for one core given its state shard [n, 132].

    All biases are folded into the weight matrices as extra contraction rows
    multiplying constant-1 activations (SN row 11; cs row 35 = 10 -> tanh = 1;
    h1[3] rows 16/48 = 1; h2 rows 64/44 = 1), so on-chip drains are pure
    copy / relu ops.
    """
    f = np.float32
    sel = lambda a: np.ascontiguousarray(a, dtype=f)

    def pad_rel(w12):  # [12, X] rel-ordered rows -> [107, X] padded cs layout
        r = np.zeros((107, w12.shape[1]), f)
        r[0:3] = w12[0:3]      # |d|
        r[32:35] = w12[3:6]    # d
        r[64:67] = w12[6:9]    # cur(=to) pos
        r[96:99] = w12[9:12]   # from pos
        return r

    watt = inputs["up_att_w"]          # [20, 64]
    wattp = pad_rel(watt[0:12])
    wattp[99:107] = watt[12:20]        # child state rest

    w1 = np.zeros((12, 64), f)
    w1[0:11] = inputs["up_fc1_w"]; w1[11] = inputs["up_fc1_b"]
    w2 = np.zeros((128, 65), f); w2[:, 0:64] = inputs["up_fc2_w"]
    b2e = np.zeros((65, 1), f); b2e[0:64, 0] = inputs["up_fc2_b"]; b2e[64, 0] = 20.0
    w3 = np.zeros((65, 64), f)
    w3[0:64] = inputs["up_fc3_w"]; w3[64] = inputs["up_fc3_b"]

    aw1, mw1 = inputs["act_l1_w"], inputs["msg_l1_w"]      # [140,400] each
    ab1, mb1 = inputs["act_l1_b"], inputs["msg_l1_b"]      # [400]
    # 832 packed l1 out cols: act[0:384] | j3(64) | msg[0:384]
    # j3: act384:400 @0:16, one-col @16, msg384:400 @32:48, one-col @48
    def pack_cols(wa, wm):
        blk = np.zeros((wa.shape[0], 64), wa.dtype)
        blk[:, 0:16] = wa[:, 384:400]
        blk[:, 32:48] = wm[:, 384:400]
        return np.concatenate([wa[:, 0:384], blk, wm[:, 0:384]], axis=1)

    wl1 = pack_cols(aw1, mw1)          # [140, 832]
    wl1a = pad_rel(wl1[0:12])          # [107, 832]
    # bias row (row 35; multiplied by tanh(10) == 1.0 in xa1)
    brow = np.zeros(832, f)
    brow[0:384] = ab1[0:384]
    brow[384:400] = ab1[384:400]
    brow[400] = 1.0                    # h1[3][16] := 1 (act l2 bias row)
    brow[416:432] = mb1[384:400]
    brow[432] = 1.0                    # h1[3][48] := 1 (msg l2 bias row)
    brow[448:832] = mb1[0:384]
    wl1a[35] = brow
    wl1b = sel(wl1[12:140])            # [128, 832]

    a2, m2 = inputs["act_l2_w"], inputs["msg_l2_w"]
    ab2, mb2 = inputs["act_l2_b"], inputs["msg_l2_b"]
    a3, m3 = inputs["act_l3_w"], inputs["msg_l3_w"]

    def l2tiles(w, b, msg_head):
        outs = {}
        for i, (r0, r1) in enumerate(((0, 128), (128, 256), (256, 384), (384, 400))):
            if i < 3:
                t = np.zeros((128, 301), f)
                t[:, 0:300] = w[r0:r1]
            else:
                if not msg_head:
                    t = np.zeros((32, 301), f)
                    t[0:16, 0:300] = w[384:400]
                    t[16, 0:300] = b
                    t[16, 300] = 1.0   # h2[44] := 1 (l3 bias row)
                else:
                    t = np.zeros((64, 301), f)
                    t[32:48, 0:300] = w[384:400]
                    t[48, 0:300] = b
                    t[48, 300] = 1.0
            outs[i] = t
        return outs

    l2a = l2tiles(a2, ab2, False)
    l2m = l2tiles(m2, mb2, True)

    im = {
        "statet": np.ascontiguousarray(shard.T, dtype=f),
        "w1": w1,
        "wattp": wattp, "wattn": -wattp,
        "nbatt": sel(-inputs["up_att_b"][:, None]),
        "w2": w2, "b2e": b2e, "w3": w3,
        "ones64": np.ones((64, 1), f), "onesb64": np.ones((1, 64), f),
        "ones128": np.ones((128, 1), f), "onesb128": np.ones((1, 128), f),
        "sel2": (np.arange(128)[:, None] // 64 == np.arange(2)[None, :]).astype(f),
        "selb2": (np.arange(128)[None, :] // 64 == np.arange(2)[:, None]).astype(f),
        "wl1a": wl1a, "wl1b": wl1b,
    }
    for i in range(4):
        im[f"wl2a{i}"] = l2a[i]
        im[f"wl2m{i}"] = l2m[i]
    for i, (r0, r1) in enumerate(((0, 128), (128, 256), (256, 300))):
        if i < 2:
            im[f"wl3a{i}"] = sel(a3[r0:r1])
            im[f"wl3m{i}"] = sel(m3[r0:r1])
        else:
            t = np.zeros((45, 1), f); t[0:44] = a3[256:300]; t[44] = inputs["act_l3_b"]
            im["wl3a2"] = t
            t = np.zeros((45, 128), f); t[0:44] = m3[256:300]; t[44] = inputs["msg_l3_b"]
            im["wl3m2"] = t
    import ml_dtypes
    for k in im:
        if k not in ("b2e", "nbatt"):
            im[k] = im[k].astype(ml_dtypes.bfloat16 if MM_DT != "f32" else f)
    return im


_CACHED_NC = None


def _run(inputs, trace=False, **kw):
    global _CACHED_NC
    if _CACHED_NC is None:
        _CACHED_NC = build_program()
    nc = _CACHED_NC
    state = np.asarray(inputs["state"], dtype=np.float32)
    n = NCH * CB
    in_maps = [pack_inputs(inputs, state[i * BLOC: i * BLOC + n]) for i in range(NCORES)]
    res = run_bass_kernel_spmd(nc, in_maps, core_ids=list(range(NCORES)),
                               trace=trace, **kw)
    outs = [np.asarray(res.results[i]["outt"]).T for i in range(NCORES)]
    return np.concatenate(outs, axis=0).astype(np.float32), res


def kernel(**inputs):
    return _run(inputs)[0]


# revision 64
# speedup vs baseline: 1.0027x; 1.0027x over previous
"""Trainium2 Bass kernel for nn_ActorGraphPolicy (tree message-passing policy).

Pure data-parallel: batch 32768 sharded across 8 NeuronCores (4096 rows each).
Per-core program processes the batch in chunks of 512 columns, with all
activations kept feature-major ([feature, batch_cols]) in SBUF so every matmul
contracts over the partition dimension.

TRN2 engine ops require 32-aligned partition bases, so concatenated inputs use
padded layouts whose pad rows carry zero weights:
  cs tile [107, CB]: |dpos|@0, dpos@32, parent_pos@64, child_state@96 (11 rows)
  l1 input: xa1 = tanh(cs[0:107]) (rel part), xa2 = [tanh(mu); tanh(msg_in)]
"""
import os
import numpy as np

import concourse.bass as bass
import concourse.tile as tile
from concourse import bacc, mybir
from concourse.bass_utils import run_bass_kernel_spmd

AF = mybir.ActivationFunctionType
OP = mybir.AluOpType
F32 = mybir.dt.float32
F32R = mybir.dt.float32r

PARENTS = [-1, 0, 0, 1, 1, 2, 2, 3, 4, 5, 6, 7]
NL, SD, MD = 12, 11, 64
CHILDREN = [[i for i, p in enumerate(PARENTS) if p == n] for n in range(NL)]
SLOT = [PARENTS[:n].count(PARENTS[n]) for n in range(NL)]  # child slot index
BATCH = 32768
NCORES = 8
BLOC = BATCH // NCORES  # 4096
CB = 512                # batch columns per chunk
EPS = 1e-12

MM_DT = os.environ.get("MM_DT", "bf16")  # 'f32' | 'bf16'
NCH = int(os.environ.get("NCH", BLOC // CB))
SN_BUFS = int(os.environ.get("SN_BUFS", 1))
BF16 = mybir.dt.bfloat16
MDT = F32 if MM_DT == "f32" else BF16  # dtype of matmul-feeding tiles/weights

UP_ORDER = list(range(NL - 1, -1, -1))
# l1 output chunk layout (816 packed cols): j0-2 act[0:384], j3 = 48 rows
# [act 384:400 | 16 zeros | msg 384:400], j4-6 msg[0:384].
M_J = [128, 128, 128, 64, 128, 128, 128]
KS4 = [128, 128, 128, 32]
H2_KS = [128, 128, 45]


def _mm_in(ap):
    return ap


def build_program(nch=NCH):
    nc = bacc.Bacc("TRN2", target_bir_lowering=False)

    def din(name, shape):
        return nc.dram_tensor(name, shape, F32, kind="ExternalInput")

    def dinm(name, shape):
        return nc.dram_tensor(name, shape, MDT, kind="ExternalInput")

    statet = dinm("statet", [132, nch * CB])
    w1 = dinm("w1", [12, 64])  # row 11 = fc1 bias (SN row 11 == 1)
    wattp = dinm("wattp", [107, 64]); wattn = dinm("wattn", [107, 64])
    nbatt = din("nbatt", [64, 1])
    w2 = dinm("w2", [128, 65]); b2e = din("b2e", [65, 1])  # col64: h2 one-row
    w3 = dinm("w3", [65, 64])  # row 64 = fc3 bias (h2 row 64 == 1)
    ones64 = dinm("ones64", [64, 1]);   onesb64 = dinm("onesb64", [1, 64])
    ones128 = dinm("ones128", [128, 1]); onesb128 = dinm("onesb128", [1, 128])
    sel2 = dinm("sel2", [128, 2]); selb2 = dinm("selb2", [2, 128])
    wl1a = dinm("wl1a", [107, 832])   # rel rows (padded); row 35 = l1 biases
    wl1b = dinm("wl1b", [128, 832])   # [mu(64); msg_in(64)] rows
    wl2a = [dinm(f"wl2a{i}", [KS4[i], 301]) for i in range(4)]
    wl2m = [dinm(f"wl2m{i}", [64 if i == 3 else KS4[i], 301]) for i in range(4)]
    wl3a = [dinm(f"wl3a{i}", [H2_KS[i], 1]) for i in range(3)]
    wl3m = [dinm(f"wl3m{i}", [H2_KS[i], 128]) for i in range(3)]
    outt = nc.dram_tensor("outt", [12, nch * CB], F32, kind="ExternalOutput")

    with tile.TileContext(nc) as tc:
        with (
            nc.allow_low_precision(reason="bf16 matmul inputs; PSUM accumulates fp32"),
            tc.tile_pool(name="wp", bufs=1) as wp,          # weights, persistent
            tc.tile_pool(name="pp", bufs=1) as pp,          # per-chunk persistent
            tc.tile_pool(name="tp", bufs=1) as tp,          # transients
            tc.tile_pool(name="pbig", bufs=4, space="PSUM") as pbig,
            tc.tile_pool(name="pnrm", bufs=2, space="PSUM") as pnrm,
            tc.tile_pool(name="psm", bufs=2, space="PSUM") as psm,
        ):
            def wload(dram, shape, tag, dt=MDT):
                t = wp.tile(shape, dt, tag=tag, name=tag)
                nc.sync.dma_start(t[:], dram[:])
                return t

            W1 = wload(w1, [12, 64], "W1")
            WATTP = wload(wattp, [107, 64], "WATTP")
            WATTN = wload(wattn, [107, 64], "WATTN")
            NBATT = wload(nbatt, [64, 1], "NBATT", dt=F32)
            W2 = wload(w2, [128, 65], "W2"); B2E = wload(b2e, [65, 1], "B2E", dt=F32)
            W3 = wload(w3, [65, 64], "W3")
            ON64 = wload(ones64, [64, 1], "ON64");  OB64 = wload(onesb64, [1, 64], "OB64")
            ON128 = wload(ones128, [128, 1], "ON128"); OB128 = wload(onesb128, [1, 128], "OB128")
            SEL2 = wload(sel2, [128, 2], "SEL2")
            ONER = wp.tile([1, CB], MDT, tag="ONER", name="ONER")
            nc.gpsimd.memset(ONER[:], 1.0)
            TENR = wp.tile([1, CB], MDT, tag="TENR", name="TENR")
            nc.gpsimd.memset(TENR[:], 10.0)
            SELB2 = wload(selb2, [2, 128], "SELB2")
            WL1A = wload(wl1a, [107, 832], "WL1A"); WL1B = wload(wl1b, [128, 832], "WL1B")
            WL2A = [wload(wl2a[i], [KS4[i], 301], f"WL2A{i}") for i in range(4)]
            WL2M = [wload(wl2m[i], [64 if i == 3 else KS4[i], 301], f"WL2M{i}")
                    for i in range(4)]
            WL3A = [wload(wl3a[i], [H2_KS[i], 1], f"WL3A{i}") for i in range(3)]
            WL3M = [wload(wl3m[i], [H2_KS[i], 128], f"WL3M{i}") for i in range(3)]

            for c in range(nch):
                # ===== input: state arrives pre-transposed ([132, B]) ========
                SN = [pp.tile([SD + 1, CB], MDT, tag=f"sn{n}", name=f"sn{n}",
                              bufs=SN_BUFS) for n in range(NL)]
                ccols = slice(c * CB, (c + 1) * CB)
                for n in range(NL):
                    nc.sync.dma_start(SN[n][0:SD, :], statet[SD * n:SD * (n + 1), ccols])
                    if c < SN_BUFS:
                        nc.sync.dma_start(SN[n][SD:SD + 1, :], ONER[:])

                MU = [pp.tile([64, CB], F32, tag=f"mu{n}", name=f"mu{n}", bufs=2) for n in range(NL)]
                MD = [pp.tile([128, CB], F32, tag=f"md{n}", name=f"md{n}") for n in range(NL)]
                CS = {}   # child -> padded cs tile [107, CB]

                def l2norm(dst_ap, raw_ap, rows, ones_t, onesb_t):
                    sq = tp.tile([rows, CB], MDT, tag="sq", name="sq")
                    nc.gpsimd.tensor_mul(sq[:], raw_ap, raw_ap)
                    pn = psm.tile([1, CB], F32, tag="S", name="pn")
                    nc.tensor.matmul(pn[:], _mm_in(ones_t[0:rows]), _mm_in(sq[:]),
                                     start=True, stop=True)
                    # 1/max(sqrt(ss), eps) == rsqrt(ss) for any nonzero vector
                    ninv = tp.tile([1, CB], MDT, tag="ninv", name="ninv")
                    nc.scalar.activation(ninv[:], pn[:], AF.Abs_reciprocal_sqrt)
                    pb = pnrm.tile([rows, CB], F32, tag="N", name="pb")
                    nc.tensor.matmul(pb[:], _mm_in(onesb_t[:, 0:rows]), _mm_in(ninv[:]),
                                     start=True, stop=True)
                    nc.vector.tensor_mul(dst_ap, raw_ap, pb[:])

                def build_cs(dst, nfrom, nto, with_rest):
                    """dst[0:3]=|d|, [32:35]=d=pos(nfrom)-pos(nto), [64:67]=pos(nto),
                    [96:96+r]=state(nfrom)."""
                    d3 = tp.tile([3, CB], MDT, tag="d3", name="d3")
                    nc.vector.tensor_sub(d3[:], SN[nfrom][0:3], SN[nto][0:3])
                    n3 = tp.tile([3, CB], MDT, tag="n3", name="n3")
                    nc.vector.tensor_scalar_mul(n3[:], d3[:], -1.0)
                    nc.vector.tensor_copy(dst[32:35], d3[:])
                    nc.vector.tensor_max(dst[0:3], d3[:], n3[:])
                    nc.vector.tensor_copy(dst[64:67], SN[nto][0:3])
                    nc.vector.tensor_copy(dst[96:96 + (SD if with_rest else 3)],
                                          SN[nfrom][0:SD if with_rest else 3])

                # ---- pre-pass: everything that depends only on state ----
                AT = {}
                for n_ in range(NL):
                    ch_ = CHILDREN[n_]
                    if not ch_:
                        continue
                    for c_i in ch_:
                        cst = pp.tile([107, CB], MDT, tag=f"cs{c_i}", name=f"cs{c_i}", bufs=2)
                        CS[c_i] = cst
                        if c < 2:
                            nc.gpsimd.memset(cst[:], 0.0)
                            nc.sync.dma_start(cst[35:36, :], TENR[:])
                        build_cs(cst, c_i, n_, True)
                    p_ = pbig.tile([64, CB], F32, tag="P", name="plpre")
                    if len(ch_) == 2:
                        nc.tensor.matmul(p_[:], _mm_in(WATTP[:]), _mm_in(CS[ch_[0]][:]),
                                         start=True, stop=False)
                        nc.tensor.matmul(p_[:], _mm_in(WATTN[:]), _mm_in(CS[ch_[1]][:]),
                                         start=False, stop=True)
                        at = tp.tile([64, CB], MDT, tag=f"at{n_}", name=f"at{n_}", bufs=2)
                        nc.scalar.activation(at[:], p_[:], AF.Sigmoid)
                    else:
                        nc.tensor.matmul(p_[:], _mm_in(WATTP[:]), _mm_in(CS[ch_[0]][:]),
                                         start=True, stop=True)
                        at = tp.tile([64, CB], MDT, tag=f"at{n_}", name=f"at{n_}", bufs=2)
                        nc.scalar.activation(at[:], p_[:], AF.Sigmoid, bias=NBATT[:])
                    AT[n_] = at
                rootcs = pp.tile([107, CB], MDT, tag="rootcs", name="rootcs", bufs=2)
                if c < 2:
                    nc.gpsimd.memset(rootcs[:], 0.0)
                    nc.sync.dma_start(rootcs[35:36, :], TENR[:])
                build_cs(rootcs, 0, NL - 1, False)

                def _up_node(n, xnp, mrp):
                    ch = CHILDREN[n]
                    r = 64 * (n % 2)
                    xm = tp.tile([128, CB], MDT, tag="xm", name="xm", bufs=2)
                    nc.scalar.activation(xm[0:64], xnp[r:r + 64], AF.Tanh)
                    if ch:
                        m = tp.tile([64, CB], F32, tag="m", name="m")
                        if len(ch) == 2:
                            # m = mu1 + sigmoid(l0 - l1) * (mu0 - mu1)
                            dmu = tp.tile([64, CB], F32, tag="dmu", name="dmu")
                            nc.gpsimd.tensor_sub(dmu[:], MU[ch[0]][:], MU[ch[1]][:])
                            nc.gpsimd.tensor_mul(dmu[:], AT[n][:], dmu[:])
                            nc.gpsimd.tensor_add(m[:], dmu[:], MU[ch[1]][:])
                        else:
                            nc.gpsimd.tensor_mul(m[:], AT[n][:], MU[ch[0]][:])
                        nc.scalar.activation(xm[64:128], m[:], AF.Tanh)
                        p2 = pbig.tile([65, CB], F32, tag="P", name="p2")
                        nc.tensor.matmul(p2[:], _mm_in(W2[:]), _mm_in(xm[:]),
                                         start=True, stop=True)
                    else:
                        p2 = pbig.tile([65, CB], F32, tag="P", name="p2")
                        nc.tensor.matmul(p2[:], _mm_in(W2[0:64]), _mm_in(xm[0:64]),
                                         start=True, stop=True)
                    h2 = tp.tile([65, CB], MDT, tag="h2u", name="h2u")
                    nc.scalar.activation(h2[:], p2[:], AF.Tanh, bias=B2E[:])
                    p3 = pbig.tile([64, CB], F32, tag="P", name="p3")
                    nc.tensor.matmul(p3[:], _mm_in(W3[:]), _mm_in(h2[:]),
                                     start=True, stop=True)
                    if mrp is not None:
                        nc.vector.tensor_copy(mrp[r:r + 64], p3[:])
                    else:
                        mr = tp.tile([64, CB], F32, tag="mr", name="mr")
                        nc.vector.tensor_copy(mr[:], p3[:])
                        l2norm(MU[n][:], mr[:], 64, ON64, OB64)

                # ================= bottom-up pass ============================
                # Nodes processed in pairs (11,10)...(1,0); x-norms and (where
                # the tree allows) mu-norms are computed 2-at-a-time through a
                # packed [128, CB] tile and a single sum/broadcast matmul pair.
                for na, nb in [(11, 10), (9, 8), (7, 6), (5, 4), (3, 2), (1, 0)]:
                    xhp = tp.tile([128, CB], F32, tag="xhp", name="xhp")
                    for n in (na, nb):
                        px = pbig.tile([64, CB], F32, tag="P", name="px")
                        nc.tensor.matmul(px[:], _mm_in(W1[:]), _mm_in(SN[n][:]),
                                         start=True, stop=True)
                        r = 64 * (n % 2)
                        nc.vector.tensor_copy(xhp[r:r + 64], px[:])
                    sqp = tp.tile([128, CB], MDT, tag="sqp", name="sqp")
                    nc.gpsimd.tensor_mul(sqp[:], xhp[:], xhp[:])
                    pn2 = psm.tile([2, CB], F32, tag="S", name="pn2")
                    nc.tensor.matmul(pn2[:], _mm_in(SEL2[:]), _mm_in(sqp[:]),
                                     start=True, stop=True)
                    xinv2 = tp.tile([2, CB], MDT, tag="xinv2", name="xinv2")
                    nc.scalar.activation(xinv2[:], pn2[:], AF.Abs_reciprocal_sqrt)
                    pb2 = pnrm.tile([128, CB], F32, tag="N", name="pb2")
                    nc.tensor.matmul(pb2[:], _mm_in(SELB2[:]), _mm_in(xinv2[:]),
                                     start=True, stop=True)
                    xnp = tp.tile([128, CB], F32, tag="xnp", name="xnp")
                    nc.vector.tensor_mul(xnp[:], xhp[:], pb2[:])

                    mu_paired = (na, nb) != (1, 0)
                    if mu_paired:
                        mrp = tp.tile([128, CB], F32, tag="mrp", name="mrp")
                    for n in (na, nb):
                        _up_node(n, xnp, mrp if mu_paired else None)
                    if mu_paired:
                        sqm = tp.tile([128, CB], MDT, tag="sqm", name="sqm")
                        nc.gpsimd.tensor_mul(sqm[:], mrp[:], mrp[:])
                        pnm = psm.tile([2, CB], F32, tag="S", name="pnm")
                        nc.tensor.matmul(pnm[:], _mm_in(SEL2[:]), _mm_in(sqm[:]),
                                         start=True, stop=True)
                        minv = tp.tile([2, CB], MDT, tag="minv", name="minv")
                        nc.scalar.activation(minv[:], pnm[:], AF.Abs_reciprocal_sqrt)
                        pbm = pnrm.tile([128, CB], F32, tag="N", name="pbm")
                        nc.tensor.matmul(pbm[:], _mm_in(SELB2[:]), _mm_in(minv[:]),
                                         start=True, stop=True)
                        for n in (na, nb):
                            r = 64 * (n % 2)
                            nc.vector.tensor_mul(MU[n][:], mrp[r:r + 64],
                                                 pbm[r:r + 64])


                # ================= top-down pass =============================
                for n in range(NL):
                    p = PARENTS[n]
                    rcs = rootcs if p < 0 else CS[n]
                    xa1 = tp.tile([107, CB], MDT, tag="xa1", name="xa1", bufs=2)
                    nc.scalar.activation(xa1[:], rcs[:], AF.Tanh)
                    xa2 = tp.tile([128, CB], MDT, tag="xa2", name="xa2", bufs=2)
                    nc.scalar.activation(xa2[0:64], MU[n][:], AF.Tanh)
                    if p >= 0:
                        mi = MD[p][64 * SLOT[n]: 64 * SLOT[n] + 64]
                        nc.scalar.activation(xa2[64:128], mi, AF.Tanh)
                        k2 = 128
                    else:
                        k2 = 64  # root: msg_in = 0 contributes nothing

                    h1 = []
                    col0 = 0
                    for j in range(7):
                        mj = M_J[j]
                        ph = pbig.tile([mj, CB], F32, tag="P", name="ph1")
                        cols = slice(col0, col0 + mj)
                        col0 += mj
                        nc.tensor.matmul(ph[:], _mm_in(WL1A[:, cols]), _mm_in(xa1[:]),
                                         start=True, stop=False)
                        nc.tensor.matmul(ph[:], _mm_in(WL1B[0:k2, cols]), _mm_in(xa2[0:k2]),
                                         start=False, stop=True)
                        h = tp.tile([mj, CB], MDT, tag=f"h1_{j}", name=f"h1_{j}", bufs=2)
                        nc.vector.tensor_single_scalar(h[:], ph[:], 0.0, OP.max)
                        h1.append(h)
                    aK = [h1[0][:], h1[1][:], h1[2][:], h1[3][0:32]]
                    mK = [h1[4][:], h1[5][:], h1[6][:], h1[3][32:64]]

                    h2a, h2m = [], []
                    for head, (Kp, WL2, acc) in enumerate(
                        ((aK, WL2A, h2a), (mK, WL2M, h2m))
                    ):
                        for i in range(3):
                            mi_ = H2_KS[i]
                            ph = pbig.tile([mi_, CB], F32, tag="P", name="ph2")
                            cols = slice(128 * i, 128 * i + mi_)  # i2: 45 cols incl one-row
                            for kk in range(4):
                                w_ap = WL2[kk][:, cols]
                                if head == 1 and kk == 3:
                                    w_ap = WL2[kk][32:64, cols]
                                nc.tensor.matmul(ph[:], _mm_in(w_ap), _mm_in(Kp[kk]),
                                                 start=(kk == 0), stop=(kk == 3))
                            h = tp.tile([mi_, CB], MDT, tag=f"h2_{head}_{i}",
                                        name=f"h2_{head}_{i}", bufs=2)
                            nc.scalar.activation(h[:], ph[:], AF.Relu)
                            acc.append(h)

                    pa = psm.tile([1, CB], F32, tag="S", name="pa")
                    for i in range(3):
                        nc.tensor.matmul(pa[:], _mm_in(WL3A[i][:]), _mm_in(h2a[i][:]),
                                         start=(i == 0), stop=(i == 2))
                    arow = tp.tile([1, CB], F32, tag="arow", name="arow")
                    nc.scalar.activation(arow[:], pa[:], AF.Tanh)
                    nc.sync.dma_start(outt[n:n + 1, ccols], arow[:])

                    pm = pbig.tile([128, CB], F32, tag="P", name="pm")
                    for i in range(3):
                        nc.tensor.matmul(pm[:], _mm_in(WL3M[i][:]), _mm_in(h2m[i][:]),
                                         start=(i == 0), stop=(i == 2))
                    mdr = tp.tile([128, CB], F32, tag="mdr", name="mdr")
                    nc.vector.tensor_copy(mdr[:], pm[:])
                    l2norm(MD[n][:], mdr[:], 128, ON128, OB128)

    nc.compile()
    return nc


def pack_inputs(inputs, shard):
    """Build the in_map for one core given its state shard [n, 132].

    All biases are folded into the weight matrices as extra contraction rows
    multiplying constant-1 activations (SN row 11; cs row 35 = 10 -> tanh = 1;
    h1[3] rows 16/48 = 1; h2 rows 64/44 = 1), so on-chip drains are pure
    copy / relu ops.
    """
    f = np.float32
    sel = lambda a: np.ascontiguousarray(a, dtype=f)

    def pad_rel(w12):  # [12, X] rel-ordered rows -> [107, X] padded cs layout
        r = np.zeros((107, w12.shape[1]), f)
        r[0:3] = w12[0:3]      # |d|
        r[32:35] = w12[3:6]    # d
        r[64:67] = w12[6:9]    # cur(=to) pos
        r[96:99] = w12[9:12]   # from pos
        return r

    watt = inputs["up_att_w"]          # [20, 64]
    wattp = pad_rel(watt[0:12])
    wattp[99:107] = watt[12:20]        # child state rest

    w1 = np.zeros((12, 64), f)
    w1[0:11] = inputs["up_fc1_w"]; w1[11] = inputs["up_fc1_b"]
    w2 = np.zeros((128, 65), f); w2[:, 0:64] = inputs["up_fc2_w"]
    b2e = np.zeros((65, 1), f); b2e[0:64, 0] = inputs["up_fc2_b"]; b2e[64, 0] = 20.0
    w3 = np.zeros((65, 64), f)
    w3[0:64] = inputs["up_fc3_w"]; w3[64] = inputs["up_fc3_b"]

    aw1, mw1 = inputs["act_l1_w"], inputs["msg_l1_w"]      # [140,400] each
    ab1, mb1 = inputs["act_l1_b"], inputs["msg_l1_b"]      # [400]
    # 832 packed l1 out cols: act[0:384] | j3(64) | msg[0:384]
    # j3: act384:400 @0:16, one-col @16, msg384:400 @32:48, one-col @48
    def pack_cols(wa, wm):
        blk = np.zeros((wa.shape[0], 64), wa.dtype)
        blk[:, 0:16] = wa[:, 384:400]
        blk[:, 32:48] = wm[:, 384:400]
        return np.concatenate([wa[:, 0:384], blk, wm[:, 0:384]], axis=1)

    wl1 = pack_cols(aw1, mw1)          # [140, 832]
    wl1a = pad_rel(wl1[0:12])          # [107, 832]
    # bias row (row 35; multiplied by tanh(10) == 1.0 in xa1)
    brow = np.zeros(832, f)
    brow[0:384] = ab1[0:384]
    brow[384:400] = ab1[384:400]
    brow[400] = 1.0                    # h1[3][16] := 1 (act l2 bias row)
    brow[416:432] = mb1[384:400]
    brow[432] = 1.0                    # h1[3][48] := 1 (msg l2 bias row)
    brow[448:832] = mb1[0:384]
    wl1a[35] = brow
    wl1b = sel(wl1[12:140])            # [128, 832]

    a2, m2 = inputs["act_l2_w"], inputs["msg_l2_w"]
    ab2, mb2 = inputs["act_l2_b"], inputs["msg_l2_b"]
    a3, m3 = inputs["act_l3_w"], inputs["msg_l3_w"]

    def l2tiles(w, b, msg_head):
        outs = {}
        for i, (r0, r1) in enumerate(((0, 128), (128, 256), (256, 384), (384, 400))):
            if i < 3:
                t = np.zeros((128, 301), f)
                t[:, 0:300] = w[r0:r1]
            else:
                if not msg_head:
                    t = np.zeros((32, 301), f)
                    t[0:16, 0:300] = w[384:400]
                    t[16, 0:300] = b
                    t[16, 300] = 1.0   # h2[44] := 1 (l3 bias row)
                else:
                    t = np.zeros((64, 301), f)
                    t[32:48, 0:300] = w[384:400]
                    t[48, 0:300] = b
                    t[48, 300] = 1.0
            outs[i] = t
        return outs

    l2a = l2tiles(a2, ab2, False)
    l2m = l2tiles(m2, mb2, True)

    im = {
        "statet": np.ascontiguousarray(shard.T, dtype=f),
        "w1": w1,
        "wattp": wattp, "wattn": -wattp,
        "nbatt": sel(-inputs["up_att_b"][:, None]),
        "w2": w2, "b2e": b2e, "w3": w3,
        "ones64": np.ones((64, 1), f), "onesb64": np.ones((1, 64), f),
        "ones128": np.ones((128, 1), f), "onesb128": np.ones((1, 128), f),
        "sel2": (np.arange(128)[:, None] // 64 == np.arange(2)[None, :]).astype(f),
        "selb2": (np.arange(128)[None, :] // 64 == np.arange(2)[:, None]).astype(f),
        "wl1a": wl1a, "wl1b": wl1b,
    }
    for i in range(4):
        im[f"wl2a{i}"] = l2a[i]
        im[f"wl2m{i}"] = l2m[i]
    for i, (r0, r1) in enumerate(((0, 128), (128, 256), (256, 300))):
        if i < 2:
            im[f"wl3a{i}"] = sel(a3[r0:r1])
            im[f"wl3m{i}"] = sel(m3[r0:r1])
        else:
            t = np.zeros((45, 1), f); t[0:44] = a3[256:300]; t[44] = inputs["act_l3_b"]
            im["wl3a2"] = t
            t = np.zeros((45, 128), f); t[0:44] = m3[256:300]; t[44] = inputs["msg_l3_b"]
            im["wl3m2"] = t
    import ml_dtypes
    for k in im:
        if k not in ("b2e", "nbatt"):
            im[k] = im[k].astype(ml_dtypes.bfloat16 if MM_DT != "f32" else f)
    return im


_CACHED_NC = None


def _run(inputs, trace=False, **kw):
    global _CACHED_NC
    if _CACHED_NC is None:
        _CACHED_NC = build_program()
    nc = _CACHED_NC
    state = np.asarray(inputs["state"], dtype=np.float32)
    n = NCH * CB
    in_maps = [pack_inputs(inputs, state[i * BLOC: i * BLOC + n]) for i in range(NCORES)]
    res = run_bass_kernel_spmd(nc, in_maps, core_ids=list(range(NCORES)),
                               trace=trace, **kw)
    outs = [np.asarray(res.results[i]["outt"]).T for i in range(NCORES)]
    return np.concatenate(outs, axis=0).astype(np.float32), res


def kernel(**inputs):
    return _run(inputs)[0]
